# revision 1
# baseline (speedup 1.0000x reference)
"""CenterLoss forward on 8 Trainium2 NeuronCores.

Reference semantics:
    distmat[b, c] = ||x_b||^2 + ||center_c||^2 - 2 <x_b, center_c>
    loss = sum(clip(distmat * onehot(labels), 1e-12, 1e12)) / B

The masked matrix is zero everywhere except (b, labels[b]), and clip() lifts
each of the B*(C-1) zeros to exactly 1e-12.  So:

    loss = ( sum_b clip(||x_b - centers[labels[b]]||^2, 1e-12, 1e12)
             + B*(C-1)*1e-12 ) / B

which needs only a row gather + per-row squared distance, not the full
(B, C) distance matrix (42 GFLOP -> ~4 MFLOP).

Device kernel (raw Bass, single basic block, SPMD data-parallel over batch):
  - centers are baked into the NEFF as a Const tensor (they are module
    *state* in the reference nn.Module); the runtime DMAs them to HBM at
    model-load time, so per-execution I/O is just the x shard + labels.
  - per core: 512 rows = 4 chunks of 128 partitions
      gpsimd:  label load, then 4 indirect-DMA row gathers
               centers[labels] -> SBUF (alternating two SWDGE queues),
               plus a tiny trailing dummy DMA that flushes the last
               gather's completion receipt through the lane promptly
      sync (HWDGE): the 4 x-chunk loads, one sem per DMA
      vector (DVE): subtract, fused square+row-reduce
               (scalar_tensor_tensor accum_out), clip
  - sync rules learned the hard way (sim race detector + hardware):
      * SWDGE/HWDGE descriptors complete out of order across rings; a
        semaphore value only proves HOW MANY of its increments landed,
        so every DMA whose completion matters gets its own semaphore
        (or a dedicated per-chunk one).
      * SWDGE sems may not be shared with HWDGE DMAs (must start at 0).
      * same-engine RAW on DVE needs an explicit sem edge.
  - per-core output: [128, 4] clipped per-row distances; host sums in
    f64, adds the analytic clip floor B*(C-1)*1e-12, divides by B.
"""

import hashlib
from contextlib import ExitStack

import numpy as np

import concourse.bass as bass
from concourse import mybir
from concourse.bass_utils import run_bass_kernel_spmd

B = 4096
D = 512
C = 10000
NCORES = 8
BL = B // NCORES          # 512 rows per core
P = 128                   # partitions
NT = BL // P              # 4 chunks per core

F32 = mybir.dt.float32
I32 = mybir.dt.int32

_CACHE = {}


def legalize_waits(nc, max_waits=1):
    """The walrus build in this container accepts at most one embedded
    sem-wait per TPB instruction ("Too many sync wait commands" otherwise).
    Split any excess into standalone single-wait InstEventSemaphore no-ops
    immediately before the instruction on the same engine — engine program
    order then enforces the identical synchronization."""
    n_split = 0
    for f in nc.m.functions:
        for b in f.blocks:
            insts = list(b.instructions)
            out = []
            for inst in insts:
                si = inst.sync_info
                waits = list(si.on_wait) if (si is not None and si.on_wait) else []
                if len(waits) > max_waits:
                    keep = waits[-max_waits:]
                    spill = waits[:-max_waits]
                    for k, w in enumerate(spill):
                        out.append(
                            mybir.InstEventSemaphore(
                                name=f"{inst.name}-lw{k}",
                                engine=inst.engine,
                                sync_info=mybir.SyncInfo(on_wait=[w], on_update=[]),
                            )
                        )
                        n_split += 1
                    inst.sync_info = mybir.SyncInfo(
                        on_wait=keep, on_update=list(si.on_update or [])
                    )
                out.append(inst)
            b.instructions = out
    return n_split


def build_nc(centers_np):
    nc = bass.Bass(num_swdge_queues=2)

    x = nc.dram_tensor("x", [BL, D], F32, kind="ExternalInput")
    # labels pre-arranged on host: [p, t] = original label[t*128 + p]
    labels = nc.dram_tensor("labels", [P, NT], I32, kind="ExternalInput")
    out = nc.dram_tensor("out", [P, NT], F32, kind="ExternalOutput")
    centers = nc.inline_tensor(
        np.ascontiguousarray(centers_np, dtype=np.float32), name="centers"
    )

    es = ExitStack()
    idx_sb = es.enter_context(nc.sbuf_tensor("idx_sb", [P, NT], I32))
    x_sb = es.enter_context(nc.sbuf_tensor("x_sb", [P, NT * D], F32))
    c_sb = es.enter_context(nc.sbuf_tensor("c_sb", [P, NT * D], F32))
    df_sb = es.enter_context(nc.sbuf_tensor("df_sb", [P, NT * D], F32))
    sq_sb = es.enter_context(nc.sbuf_tensor("sq_sb", [P, NT * D], F32))
    dcols = es.enter_context(nc.sbuf_tensor("dcols", [P, NT], F32))
    dclip = es.enter_context(nc.sbuf_tensor("dclip", [P, NT], F32))
    scr_sb = es.enter_context(nc.sbuf_tensor("scr_sb", [P, NT], I32))
    scr2_sb = es.enter_context(nc.sbuf_tensor("scr2_sb", [P, NT], I32))
    idx_sem = es.enter_context(nc.semaphore("idx_sem"))
    c_sems = [es.enter_context(nc.semaphore(f"c_sem{t}")) for t in range(NT)]
    xc_sems = [es.enter_context(nc.semaphore(f"xc_sem{t}")) for t in range(NT)]
    v_sem = es.enter_context(nc.semaphore("v_sem"))
    o_sem = es.enter_context(nc.semaphore("o_sem"))
    dve_sem = es.enter_context(nc.semaphore("dve_sem"))
    f_sem = es.enter_context(nc.semaphore("f_sem"))

    # ---- gpsimd: labels, then the gathers ----
    nc.gpsimd.dma_start(out=idx_sb[:, :], in_=labels[:, :]).then_inc(idx_sem, 16)
    # dummy DMA right behind the label load: the lane processes it next,
    # which pushes the label DMA's completion receipt through promptly
    # (otherwise idx_sem fires ~2 us late while the lane idles)
    nc.gpsimd.dma_start(out=scr2_sb[:, :], in_=labels[:, :]).then_inc(f_sem, 16)
    # ---- sync/HWDGE: x chunks in parallel with the above ----
    for t in range(NT):
        nc.sync.dma_start(
            out=x_sb[:, t * D:(t + 1) * D], in_=x[t * P:(t + 1) * P, :]
        ).then_inc(xc_sems[t], 16)
    nc.gpsimd.wait_ge(idx_sem, 16)  # indices resident before gathers
    gather_insts = []
    for t in range(NT):
        gi = nc.gpsimd.indirect_dma_start(
            out=c_sb[:, t * D:(t + 1) * D],
            out_offset=None,
            in_=centers[:],
            in_offset=bass.IndirectOffsetOnAxis(ap=idx_sb[:, t:t + 1], axis=0),
        ).then_inc(c_sems[t], 16)
        gather_insts.append(gi)
    # trailing dummy SWDGE DMA: flushes the last gather's completion receipt
    nc.gpsimd.dma_start(out=scr_sb[:, :], in_=labels[:, :]).then_inc(f_sem, 16)

    # ---- vector: per-chunk subtract + fused square/row-reduce ----
    n_dve = 0
    for t in range(NT):
        cs = slice(t * D, (t + 1) * D)
        nc.vector.wait_ge(xc_sems[t], 16)
        nc.vector.wait_ge(c_sems[t], 16)
        nc.vector.tensor_tensor(
            out=df_sb[:, cs],
            in0=x_sb[:, cs],
            in1=c_sb[:, cs],
            op=mybir.AluOpType.subtract,
        ).then_inc(dve_sem, 1)
        n_dve += 1
        nc.vector.wait_ge(dve_sem, n_dve)
        nc.vector.scalar_tensor_tensor(
            out=sq_sb[:, cs],
            in0=df_sb[:, cs],
            scalar=1.0,
            in1=df_sb[:, cs],
            op0=mybir.AluOpType.mult,
            op1=mybir.AluOpType.mult,
            accum_out=dcols[:, t:t + 1],
        ).then_inc(dve_sem, 1)
        n_dve += 1
    nc.vector.wait_ge(dve_sem, n_dve)
    nc.vector.tensor_scalar(
        out=dclip[:, :],
        in0=dcols[:, :],
        scalar1=1e-12,
        scalar2=1e12,
        op0=mybir.AluOpType.max,
        op1=mybir.AluOpType.min,
    ).then_inc(v_sem, 1)

    # ---- result out; runtime drains rings before reading outputs ----
    nc.gpsimd.wait_ge(v_sem, 1)
    nc.gpsimd.dma_start(out=out[:, :], in_=dclip[:, :]).then_inc(o_sem, 16)

    # alternate gathers across the two SWDGE queues
    for t, gi in enumerate(gather_insts):
        if t % 2 == 1:
            gi.ins.queue = "qPoolDynamic1"

    # NOTE: the ExitStack is intentionally NOT closed — closing would free
    # the semaphores and emit an expensive end-of-program drain + barrier;
    # Bass already clears the whole sem range in its preamble, so repeated
    # executions stay safe without it.
    legalize_waits(nc)
    return nc


def _get_nc(centers_np):
    arr = np.ascontiguousarray(centers_np, np.float32)
    key = hashlib.md5(arr.tobytes()).hexdigest()
    if _CACHE.get("key") != key:
        _CACHE["nc"] = build_nc(arr)
        _CACHE["key"] = key
    return _CACHE["nc"]


def make_in_maps(x, labels, centers=None):
    x = np.ascontiguousarray(np.asarray(x, dtype=np.float32))
    # [p, t] = label[t*128 + p] within each core's 512-row shard
    labels_i32 = np.ascontiguousarray(
        np.asarray(labels).astype(np.int32).reshape(NCORES, NT, P).transpose(0, 2, 1)
    )
    xs = x.reshape(NCORES, BL, D)
    return [{"x": xs[i], "labels": labels_i32[i]} for i in range(NCORES)]


def finalize(results):
    total = 0.0
    for r in results:
        total += float(np.asarray(r["out"], dtype=np.float64).sum())
    loss = (total + B * (C - 1) * 1e-12) / B
    return np.array(loss, dtype=np.float32)


def kernel(x, labels, centers):
    nc = _get_nc(centers)
    in_maps = make_in_maps(x, labels)
    res = run_bass_kernel_spmd(nc, in_maps, core_ids=list(range(NCORES)))
    return finalize(res.results)



# revision 9
# speedup vs baseline: 1.1413x; 1.1413x over previous
"""CenterLoss forward on 8 Trainium2 NeuronCores.

Reference semantics:
    distmat[b, c] = ||x_b||^2 + ||center_c||^2 - 2 <x_b, center_c>
    loss = sum(clip(distmat * onehot(labels), 1e-12, 1e12)) / B

The masked matrix is zero everywhere except (b, labels[b]), and clip() lifts
each of the B*(C-1) zeros to exactly 1e-12.  So:

    loss = ( sum_b clip(||x_b - centers[labels[b]]||^2, 1e-12, 1e12)
             + B*(C-1)*1e-12 ) / B

which needs only a row gather + per-row squared distance, not the full
(B, C) distance matrix (42 GFLOP -> ~4 MFLOP).

Device kernel (raw Bass, single basic block, SPMD data-parallel over batch),
v2 — tuned from the v1 trace (23.6 us):
  - everything in bf16 (x shard, baked centers table): halves DMA bytes and
    doubles DVE throughput; quantization error ~0.2%/element, random sign,
    averages out over the 512-d row sums (tolerance is 2e-2).
  - labels are loaded FIRST, via the sync engine's HWDGE (625ns issue)
    instead of gpsimd SWDGE — the label->gather dependency is the critical
    path prefix, and v1 lost ~2us issuing it late from gpsimd.
  - x shard is pre-transposed on host to [128 partitions, NT*D] so it loads
    as ONE direct DMA (128 x 4KB descriptors) instead of four chunked ones.
  - the row gather centers[labels] runs as TWO indirect SWDGE DMAs of 256
    rows each (offset AP [128, 2]) instead of four of 128: SWDGE issue cost
    is 994ns fixed + 0.34ns/descriptor, so fewer instructions win; two (not
    one) keeps the first half overlappable with compute.  A trailing dummy
    SWDGE DMA flushes the last gather's completion receipt (observed ~2us
    receipt lag otherwise).
  - compute is split across engines: DVE does bf16 subtract per 128-row
    subtile, ACT does Square-activation with row accumulate (accum_out) into
    an f32 column; the ACT Square table is primed at program start so the
    1.28us table load hides under the DMAs.  DVE then clips all 512 row
    distances in one [128, 4] tensor_scalar.
  - output [128, 4] f32 goes out via sync HWDGE (v1 paid a 786ns gpsimd
    DRAIN + 675ns SWDGE issue here).
  - sync rules (sim race detector + hardware, inherited from v1):
      * every DMA whose completion matters gets its own semaphore.
      * SWDGE sems may not be shared with HWDGE DMAs.
      * same-engine RAW on DVE needs an explicit sem edge; cross-engine
        (DVE->ACT) edges via dve_sem counts.
  - per-core output: [128, 4] clipped per-row distances; host sums in
    f64, adds the analytic clip floor B*(C-1)*1e-12, divides by B.
"""

import hashlib
from contextlib import ExitStack

import ml_dtypes
import numpy as np

import concourse.bass as bass
from concourse import mybir
from concourse.bass_utils import run_bass_kernel_spmd

B = 4096
D = 512
C = 10000
NCORES = 8
BL = B // NCORES          # 512 rows per core
P = 128                   # partitions
NT = BL // P              # 4 subtiles of 128 rows per core
# one indirect gather per 128-row subtile: multi-column offset APs ([128,2])
# both mis-pair descriptors (dest contiguous runs of 2 rows swallow the
# second offset) and fall off the fast Q7 descriptor-gen path (~167ns/desc)
NG = NT
TPG = 1

F32 = mybir.dt.float32
BF16 = mybir.dt.bfloat16
I32 = mybir.dt.int32

_CACHE = {}


def legalize_waits(nc, max_waits=1):
    """The walrus build in this container accepts at most one embedded
    sem-wait per TPB instruction ("Too many sync wait commands" otherwise).
    Split any excess into standalone single-wait InstEventSemaphore no-ops
    immediately before the instruction on the same engine — engine program
    order then enforces the identical synchronization."""
    n_split = 0
    for f in nc.m.functions:
        for b in f.blocks:
            insts = list(b.instructions)
            out = []
            for inst in insts:
                si = inst.sync_info
                waits = list(si.on_wait) if (si is not None and si.on_wait) else []
                if len(waits) > max_waits:
                    keep = waits[-max_waits:]
                    spill = waits[:-max_waits]
                    for k, w in enumerate(spill):
                        out.append(
                            mybir.InstEventSemaphore(
                                name=f"{inst.name}-lw{k}",
                                engine=inst.engine,
                                sync_info=mybir.SyncInfo(on_wait=[w], on_update=[]),
                            )
                        )
                        n_split += 1
                    inst.sync_info = mybir.SyncInfo(
                        on_wait=keep, on_update=list(si.on_update or [])
                    )
                out.append(inst)
            b.instructions = out
    return n_split


def build_nc(centers_np):
    nc = bass.Bass(num_swdge_queues=2)

    # host pre-arranges x: [p, t*D + d] = x_core[t*128 + p, d], bf16
    x = nc.dram_tensor("x", [P, NT * D], BF16, kind="ExternalInput")
    # labels pre-arranged on host: [p, t] = original label[t*128 + p]
    labels = nc.dram_tensor("labels", [P, NT], I32, kind="ExternalInput")
    out = nc.dram_tensor("out", [P, NT], F32, kind="ExternalOutput")
    centers = nc.inline_tensor(
        np.ascontiguousarray(centers_np.astype(ml_dtypes.bfloat16)), name="centers"
    )

    es = ExitStack()
    idx_sb = es.enter_context(nc.sbuf_tensor("idx_sb", [P, NT], I32))
    # all tiles 2D: a 3D dest AP on the indirect DMA lowers to a 3-dim
    # physical AP whose extra unit dim knocks the SWDGE off its fast path
    # (~20us per DMA, transfer deferred until a poll timeout)
    x_sb = es.enter_context(nc.sbuf_tensor("x_sb", [P, NT * D], BF16))
    c_sb = es.enter_context(nc.sbuf_tensor("c_sb", [P, NT * D], BF16))
    df_sb = es.enter_context(nc.sbuf_tensor("df_sb", [P, NT * D], BF16))
    sq_sb = es.enter_context(nc.sbuf_tensor("sq_sb", [P, D], BF16))
    prime_sb = es.enter_context(nc.sbuf_tensor("prime_sb", [P, 1], BF16))
    dcol = es.enter_context(nc.sbuf_tensor("dcol", [P, NT], F32))
    dclip = es.enter_context(nc.sbuf_tensor("dclip", [P, NT], F32))
    scr_sb = es.enter_context(nc.sbuf_tensor("scr_sb", [P, NT], I32))

    lbl_sem = es.enter_context(nc.semaphore("lbl_sem"))
    x_sem = es.enter_context(nc.semaphore("x_sem"))
    g_sems = [es.enter_context(nc.semaphore(f"g_sem{g}")) for g in range(NG)]
    f_sem = es.enter_context(nc.semaphore("f_sem"))
    dve_sem = es.enter_context(nc.semaphore("dve_sem"))
    act_sem = es.enter_context(nc.semaphore("act_sem"))
    v_sem = es.enter_context(nc.semaphore("v_sem"))
    o_sem = es.enter_context(nc.semaphore("o_sem"))

    # ---- sync/HWDGE: labels first (critical-path prefix), then x ----
    nc.sync.dma_start(out=idx_sb[:, :], in_=labels[:, :]).then_inc(lbl_sem, 16)
    nc.sync.dma_start(out=x_sb[:, :], in_=x[:, :]).then_inc(x_sem, 16)

    # ---- scalar/ACT: prime the Square activation table under the DMAs ----
    nc.scalar.activation(
        out=prime_sb[:, :], in_=prime_sb[:, :],
        func=mybir.ActivationFunctionType.Square,
    )

    # ---- gpsimd: the gathers, as soon as the indices land ----
    nc.gpsimd.wait_ge(lbl_sem, 16)
    gather_insts = []
    for g in range(NG):
        gi = nc.gpsimd.indirect_dma_start(
            out=c_sb[:, g * D:(g + 1) * D],
            out_offset=None,
            in_=centers[:],
            in_offset=bass.IndirectOffsetOnAxis(ap=idx_sb[:, g:g + 1], axis=0),
        ).then_inc(g_sems[g], 16)
        gather_insts.append(gi)
    # trailing dummy SWDGE DMA: flushes the last gather's completion receipt
    nc.gpsimd.dma_start(out=scr_sb[:, :], in_=labels[:, :]).then_inc(f_sem, 16)
    # alternate gathers across the two SWDGE queues
    for g, gi in enumerate(gather_insts):
        if g % 2 == 1:
            gi.ins.queue = "qPoolDynamic1"

    # ---- vector: bf16 subtract per subtile; scalar: Square + row-accum ----
    nc.vector.wait_ge(x_sem, 16)
    n_dve = 0
    for t in range(NT):
        cs = slice(t * D, (t + 1) * D)
        nc.vector.wait_ge(g_sems[t], 16)
        nc.vector.tensor_tensor(
            out=df_sb[:, cs],
            in0=x_sb[:, cs],
            in1=c_sb[:, cs],
            op=mybir.AluOpType.subtract,
        ).then_inc(dve_sem, 1)
        n_dve += 1
        nc.scalar.wait_ge(dve_sem, n_dve)
        nc.scalar.activation(
            out=sq_sb[:, :],
            in_=df_sb[:, cs],
            func=mybir.ActivationFunctionType.Square,
            accum_out=dcol[:, t:t + 1],
        ).then_inc(act_sem, 1)

    # ---- vector: clip all 512 row distances at once ----
    nc.vector.wait_ge(act_sem, NT)
    nc.vector.tensor_scalar(
        out=dclip[:, :],
        in0=dcol[:, :],
        scalar1=1e-12,
        scalar2=1e12,
        op0=mybir.AluOpType.max,
        op1=mybir.AluOpType.min,
    ).then_inc(v_sem, 1)

    # ---- result out via sync HWDGE; runtime drains rings before reading ----
    nc.sync.wait_ge(v_sem, 1)
    nc.sync.dma_start(out=out[:, :], in_=dclip[:, :]).then_inc(o_sem, 16)

    # NOTE: the ExitStack is intentionally NOT closed — closing would free
    # the semaphores and emit an expensive end-of-program drain + barrier;
    # Bass already clears the whole sem range in its preamble, so repeated
    # executions stay safe without it.
    legalize_waits(nc)
    return nc


def _get_nc(centers_np):
    arr = np.ascontiguousarray(centers_np, np.float32)
    key = hashlib.md5(arr.tobytes()).hexdigest()
    if _CACHE.get("key") != key:
        _CACHE["nc"] = build_nc(arr)
        _CACHE["key"] = key
    return _CACHE["nc"]


def make_in_maps(x, labels, centers=None):
    xb = np.asarray(x, dtype=np.float32).astype(ml_dtypes.bfloat16)
    # [p, t*D + d] = x[core*512 + t*128 + p, d]
    xb = np.ascontiguousarray(
        xb.reshape(NCORES, NT, P, D).transpose(0, 2, 1, 3).reshape(NCORES, P, NT * D)
    )
    # [p, t] = label[t*128 + p] within each core's 512-row shard
    labels_i32 = np.ascontiguousarray(
        np.asarray(labels).astype(np.int32).reshape(NCORES, NT, P).transpose(0, 2, 1)
    )
    return [{"x": xb[i], "labels": labels_i32[i]} for i in range(NCORES)]


def finalize(results):
    total = 0.0
    for r in results:
        total += float(np.asarray(r["out"], dtype=np.float64).sum())
    loss = (total + B * (C - 1) * 1e-12) / B
    return np.array(loss, dtype=np.float32)


def kernel(x, labels, centers):
    nc = _get_nc(centers)
    in_maps = make_in_maps(x, labels)
    res = run_bass_kernel_spmd(nc, in_maps, core_ids=list(range(NCORES)))
    return finalize(res.results)
